# revision 14
# baseline (speedup 1.0000x reference)
"""Trainium2 Bass kernel: sparse multi-head 3x3x3 voxel conv (gnn message passing).

v3: all batched DMA via the `dma_gather` extended instruction.
- Host reconstructs exact voxel linear ids from the kernel_map graph (BFS per
  connected component) and renumbers points spatially -> neighbor rank
  distance <= ~1700, so int16 gather windows work.
- Per core: 17 subpieces of 6000 dests. Per sub: one 32768-row feat window,
  k-grouped gather chunks (3 dma_gather calls of 8192 rows), per-chunk
  transpose+matmul -> H rows (256B, chunk-major, per-partition contiguous) in
  a DRAM scratch block of <=32768 rows; fold gathers slots per count-sorted
  dest tile via dma_gather from the sub's H block and reduces on DVE (f32).
"""

import sys
from contextlib import ExitStack

for p in ("/opt/trn_rl_repo", "/root/.axon_site/_ro/trn_rl_repo"):
    if p not in sys.path:
        sys.path.insert(0, p)

import numpy as np
import ml_dtypes

import concourse.tile as tile
from concourse import bass, bacc, mybir
from concourse.masks import make_identity

BF16 = ml_dtypes.bfloat16
C = 64
CHW = 128        # padded table/H row channels (256B rows)
NH = 4
CHH = 16
KVOL = 27
SIDE = 200
SUB = 6000       # dests per subpiece
MARG = 6000      # core table margin (max neighbor rank distance ~1700)
WIN = 32768
QG = 64          # chunks per gather call == H-group size
MF = 64          # max fold slot-columns per call
TMAX = 16


def cdiv(a, b):
    return (a + b - 1) // b


def spatial_order(kernel_map):
    """Exact relative voxel linear ids from the 27-neighborhood graph."""
    from scipy.sparse import coo_matrix
    from scipy.sparse.csgraph import connected_components
    N = kernel_map.shape[1]
    deltas = np.array([dx * SIDE * SIDE + dy * SIDE + dz
                       for dx in (-1, 0, 1) for dy in (-1, 0, 1) for dz in (-1, 0, 1)],
                      dtype=np.int64)
    rows = []
    cols = []
    for k in range(KVOL):
        if k == 13:
            continue
        m = kernel_map[k] >= 0
        rows.append(np.nonzero(m)[0])
        cols.append(kernel_map[k][m])
    g = coo_matrix((np.ones(sum(len(r) for r in rows), np.int8),
                    (np.concatenate(rows), np.concatenate(cols))), shape=(N, N))
    ncomp, labels = connected_components(g, directed=False)
    lin = np.zeros(N, np.int64)
    known = np.zeros(N, bool)
    _, seed_idx = np.unique(labels, return_index=True)
    lin[seed_idx] = labels[seed_idx].astype(np.int64) * (1 << 24)
    known[seed_idx] = True
    frontier = seed_idx
    while len(frontier):
        new = []
        for k in range(KVOL):
            if k == 13:
                continue
            nbr = kernel_map[k][frontier]
            ok = nbr >= 0
            nbr_v = nbr[ok]
            unk = ~known[nbr_v]
            tgt = nbr_v[unk]
            if len(tgt) == 0:
                continue
            lin[tgt] = lin[frontier[ok][unk]] + deltas[k]
            known[tgt] = True
            new.append(tgt)
        frontier = np.unique(np.concatenate(new)) if new else np.array([], np.int64)
    assert known.all()
    return np.argsort(lin, kind="stable")


def pack_idx(idx):
    """[n] int -> [128, n/16] int16: j at (j%16, j//16), replicated to 8 q7 cores."""
    n = len(idx)
    assert n % 16 == 0
    t = np.zeros((16, n // 16), np.int16)
    t[np.arange(n) % 16, np.arange(n) // 16] = idx.astype(np.int16)
    return np.tile(t, (8, 1))


def host_prep(feats, weight, kernel_map, n_cores):
    feats = np.asarray(feats)
    weight = np.asarray(weight)
    kernel_map = np.asarray(kernel_map)
    N = kernel_map.shape[1]
    S = N // n_cores
    NSUB = cdiv(S, SUB)
    CORE_ROWS = (NSUB - 1) * SUB + WIN

    order = spatial_order(kernel_map)
    inv = np.empty(N, np.int64)
    inv[order] = np.arange(N)
    feats_r = feats[order]
    kmr = np.where(kernel_map[:, order] >= 0,
                   inv[np.maximum(kernel_map[:, order], 0)], -1).astype(np.int64)

    w_sb = np.zeros((128, KVOL * C), dtype=BF16)
    for k in range(KVOL):
        blk = np.zeros((C, C), np.float32)
        for h in range(NH):
            blk[h * CHH:(h + 1) * CHH, h * CHH:(h + 1) * CHH] = weight[k, h]
        w_sb[:C, k * C:(k + 1) * C] = blk.astype(BF16)
        w_sb[C:, k * C:(k + 1) * C] = w_sb[:C, k * C:(k + 1) * C]

    # shared structure: per (sub, k) chunk counts = max over cores
    sub_lens = [min(SUB, S - s * SUB) for s in range(NSUB)]
    # per core, per sub, per k: (local dest idx within sub, local src row in core table)
    core_sub_runs = [[None] * NSUB for _ in range(n_cores)]
    for c in range(n_cores):
        tab_lo = c * S - MARG
        for s in range(NSUB):
            lo = c * S + s * SUB
            hi = lo + sub_lens[s]
            runs = []
            for k in range(KVOL):
                col = kmr[k, lo:hi]
                m = col >= 0
                d_loc = np.nonzero(m)[0]
                src_loc = col[m] - tab_lo
                runs.append((d_loc, src_loc))
            core_sub_runs[c][s] = runs

    n_chunks = np.zeros((NSUB, KVOL), np.int64)
    for s in range(NSUB):
        for k in range(KVOL):
            n_chunks[s][k] = max(cdiv(len(core_sub_runs[c][s][k][0]), 128)
                                 for c in range(n_cores))

    # per sub: chunk layout (k-major), zero chunk; uniform HCH across subs
    sub_meta = []
    for s in range(NSUB):
        chunk_k = []
        chunk_start_k = []
        for k in range(KVOL):
            chunk_start_k.append(len(chunk_k))
            chunk_k.extend([k] * int(n_chunks[s][k]))
        HCH_REAL = len(chunk_k)
        sub_meta.append(dict(chunk_k=chunk_k, chunk_start_k=chunk_start_k,
                             HCH_REAL=HCH_REAL, ZC=HCH_REAL))
    HCH = cdiv(max(m["HCH_REAL"] for m in sub_meta) + 1, QG) * QG
    assert HCH <= 256, HCH
    for m in sub_meta:
        m["HCH"] = HCH

    # fold structure per sub: count-sorted tiles, R_t = max over cores
    for s in range(NSUB):
        L = sub_lens[s]
        n_tiles = cdiv(L, 128)
        counts_sorted = np.zeros((n_cores, L), np.int64)
        for c in range(n_cores):
            counts = np.zeros(L, np.int64)
            for k in range(KVOL):
                counts[core_sub_runs[c][s][k][0]] += 1
            counts_sorted[c] = -np.sort(-counts)
        R_t = [max(1, int(counts_sorted[:, t * 128].max())) for t in range(n_tiles)]
        assert R_t[0] <= MF
        col_base = np.concatenate([[0], np.cumsum(R_t)]).astype(np.int64)
        calls = []
        t = 0
        while t < n_tiles:
            R = R_t[t]
            tb = t
            while tb < n_tiles and R_t[tb] == R:
                tb += 1
            T = max(1, min(TMAX, MF // R))
            while t < tb:
                Tc = min(T, tb - t)
                calls.append((t, Tc, R, int(col_base[t])))
                t += Tc
        sub_meta[s].update(n_tiles=n_tiles, R_t=R_t, col_base=col_base,
                           NR=int(col_base[-1]), calls=calls, L=L)

    tile_base = np.concatenate([[0], np.cumsum([m["n_tiles"] for m in sub_meta])])
    gidx_base = np.concatenate([[0], np.cumsum([m["HCH"] * 128 for m in sub_meta])])
    fidx_base = np.concatenate([[0], np.cumsum([m["NR"] * 128 for m in sub_meta])])
    GT = int(gidx_base[-1])
    FT = int(fidx_base[-1])
    n_tiles_tot = int(tile_base[-1])

    meta = dict(N=N, S=S, NSUB=NSUB, CORE_ROWS=CORE_ROWS, sub_lens=sub_lens,
                sub_meta=sub_meta, tile_base=tile_base, gidx_base=gidx_base,
                fidx_base=fidx_base, GT=GT, FT=FT, n_tiles_tot=n_tiles_tot)

    in_maps = []
    perms = []   # per core: global out row (in renumbered space) per (sub tile rank)
    for c in range(n_cores):
        tab_lo = c * S - MARG
        table = np.zeros((CORE_ROWS, CHW), dtype=BF16)
        glo = max(0, tab_lo)
        ghi = min(N, tab_lo + CORE_ROWS)
        table[glo - tab_lo:ghi - tab_lo, :C] = feats_r[glo:ghi].astype(BF16)

        gidx = np.zeros(GT, np.int64)
        fidx = np.zeros(FT, np.int64)
        core_perm = []
        for s in range(NSUB):
            sm = sub_meta[s]
            HCH = sm["HCH"]
            base_s = s * SUB
            runs = core_sub_runs[c][s]
            # gather idxs + slot map: contribution i of (k) -> slot (col, p)
            g_sub = np.zeros(HCH * 128, np.int64)   # idx j = col*128 + p
            slot_of = {}
            all_d = []
            all_slot = []
            for k in range(KVOL):
                d_loc, src_loc = runs[k]
                Lk = len(d_loc)
                if Lk:
                    j = np.arange(Lk)
                    cols = sm["chunk_start_k"][k] + j // 128
                    ps = j % 128
                    rel = src_loc - base_s
                    assert rel.min() >= 0 and rel.max() < WIN, (c, s, k, rel.min(), rel.max())
                    g_sub[cols * 128 + ps] = rel
                    all_d.append(d_loc)
                    all_slot.append(ps * HCH + cols)   # h row id
            gidx[gidx_base[s]:gidx_base[s + 1]] = g_sub

            # fold: count-sort dests within sub (per core), fill fold idxs
            L = sm["L"]
            counts = np.zeros(L, np.int64)
            ad = np.concatenate(all_d)
            aslot = np.concatenate(all_slot)
            np.add.at(counts, ad, 1)
            order_sub = np.argsort(-counts, kind="stable")
            rank = np.empty(L, np.int64)
            rank[order_sub] = np.arange(L)
            pr = rank[ad]
            o2 = np.argsort(pr, kind="stable")
            sr = pr[o2]
            sh = aslot[o2]
            grp_start = np.searchsorted(sr, np.arange(L))
            r_idx = np.arange(len(sr)) - grp_start[sr]
            t_of = sr // 128
            p_of = sr % 128
            R_arr = np.array(sm["R_t"])
            assert (r_idx < R_arr[t_of]).all()
            col = sm["col_base"][t_of] + r_idx
            f_sub = np.empty(sm["NR"] * 128, np.int64)
            # pads -> zero chunk row of same partition: p*HCH + ZC
            pcol = np.arange(sm["NR"] * 128)
            f_sub[:] = (pcol % 128) * HCH + sm["ZC"]
            f_sub[col * 128 + p_of] = sh
            fidx[fidx_base[s]:fidx_base[s + 1]] = f_sub
            core_perm.append(c * S + base_s + order_sub)

        in_maps.append({
            "table": table,
            "w_sb": w_sb,
            "gidx": pack_idx(gidx),
            "fidx": pack_idx(fidx),
        })
        perms.append(core_perm)

    return in_maps, perms, meta, order


def build_program(n_cores, meta):
    import os
    KSUBS = int(os.environ.get("KSUBS", "0")) or None      # limit #subs
    KNOFOLD = os.environ.get("KNOFOLD", "0") == "1"        # skip fold phase
    KNOGATH = os.environ.get("KNOGATH", "0") == "1"        # skip gather+compute
    NSUB = meta["NSUB"]
    sub_meta = meta["sub_meta"]
    CORE_ROWS = meta["CORE_ROWS"]
    n_tiles_tot = meta["n_tiles_tot"]

    nc = bacc.Bacc("TRN2", target_bir_lowering=False, debug=False,
                   num_devices=n_cores)

    table = nc.dram_tensor("table", [CORE_ROWS, CHW], mybir.dt.bfloat16,
                           kind="ExternalInput").ap()
    w_in = nc.dram_tensor("w_sb", [128, KVOL * C], mybir.dt.bfloat16,
                          kind="ExternalInput").ap()
    gidx_d = nc.dram_tensor("gidx", [128, meta["GT"] // 16], mybir.dt.int16,
                            kind="ExternalInput").ap()
    fidx_d = nc.dram_tensor("fidx", [128, meta["FT"] // 16], mybir.dt.int16,
                            kind="ExternalInput").ap()
    out = nc.dram_tensor("out", [128, n_tiles_tot * C], mybir.dt.float32,
                         kind="ExternalOutput").ap()

    with tile.TileContext(nc) as tc, ExitStack() as ctx:
        dram = ctx.enter_context(tc.tile_pool(name="dram", bufs=2, space="DRAM"))

        wpool = ctx.enter_context(tc.tile_pool(name="w", bufs=1))
        w_t = wpool.tile([128, KVOL * C], mybir.dt.bfloat16)
        nc.sync.dma_start(out=w_t[:], in_=w_in[:])
        ident = wpool.tile([128, 128], mybir.dt.bfloat16)
        make_identity(nc, ident[:])

        gip = ctx.enter_context(tc.tile_pool(name="gi", bufs=3))
        fip = ctx.enter_context(tc.tile_pool(name="fi", bufs=3))
        gp = ctx.enter_context(tc.tile_pool(name="G", bufs=2))
        xp = ctx.enter_context(tc.tile_pool(name="X", bufs=8))
        hp = ctx.enter_context(tc.tile_pool(name="H", bufs=2))
        sp = ctx.enter_context(tc.tile_pool(name="slots", bufs=2))
        op = ctx.enter_context(tc.tile_pool(name="outp", bufs=3))
        psx = ctx.enter_context(tc.tile_pool(name="psx", bufs=4, space="PSUM"))
        psh = ctx.enter_context(tc.tile_pool(name="psh", bufs=4, space="PSUM"))

        for s in range(NSUB if KSUBS is None else min(NSUB, KSUBS)):
            sm = sub_meta[s]
            HCH = sm["HCH"]
            chunk_k = sm["chunk_k"]
            HCH_REAL = sm["HCH_REAL"]
            base_s = s * SUB

            h_sub = dram.tile([128 * HCH, CHW], mybir.dt.bfloat16)

            # groups entirely past the zero chunk are never referenced
            g_last = cdiv(HCH_REAL + 1, QG)
            for q0 in ([] if KNOGATH else range(0, g_last * QG, QG)):
                gi = gip.tile([128, QG * 8], mybir.dt.int16)
                c0 = (int(meta["gidx_base"][s]) + q0 * 128) // 16
                nc.sync.dma_start(out=gi[:], in_=gidx_d[:, c0:c0 + QG * 8])
                gbuf = gp.tile([128, QG * CHW], mybir.dt.bfloat16)
                nc.gpsimd.dma_gather(
                    out_ap=gbuf[:].rearrange("p (m c) -> p m c", c=CHW),
                    in_ap=table[base_s:base_s + WIN, :],
                    idxs_ap=gi[:],
                    num_idxs=QG * 128,
                    num_idxs_reg=QG * 128,
                    elem_size=CHW,
                    single_packet=False,
                )
                h_t = hp.tile([128, QG * CHW], mybir.dt.bfloat16)
                if q0 + QG > HCH_REAL:
                    # group contains pad/zero chunks: zero the whole tile so
                    # fold pads (and H-write reads) see defined zeros
                    nc.vector.memset(h_t[:], 0.0)
                for q in range(QG):
                    colc = q0 + q
                    if colc >= HCH_REAL:
                        continue
                    k = chunk_k[colc]
                    x_ps = psx.tile([64, 128], mybir.dt.bfloat16)
                    nc.tensor.transpose(
                        out=x_ps[:], in_=gbuf[:, q * CHW:q * CHW + C],
                        identity=ident[:])
                    x_t = xp.tile([64, 128], mybir.dt.bfloat16)
                    nc.vector.tensor_copy(out=x_t[:], in_=x_ps[:])
                    h_ps = psh.tile([128, C], mybir.dt.float32)
                    nc.tensor.matmul(
                        out=h_ps[:],
                        lhsT=x_t[:],
                        rhs=w_t[0:64, k * C:(k + 1) * C],
                        start=True, stop=True,
                    )
                    nc.scalar.activation(
                        h_t[:, q * CHW:q * CHW + C], h_ps[:],
                        mybir.ActivationFunctionType.Copy,
                    )
                # H rows: slot (col, p) -> row p*HCH + col; this group: cols [q0, q0+QG)
                nc.sync.dma_start(
                    out=h_sub[:].rearrange(
                        "(p q) c -> p q c", q=HCH)[:, q0:q0 + QG, :],
                    in_=h_t[:].rearrange("p (q c) -> p q c", c=CHW),
                )

            for (t0, T, R, col0) in ([] if KNOFOLD else sm["calls"]):
                ncols = T * R
                fi = fip.tile([128, MF * 8], mybir.dt.int16)
                c0 = (int(meta["fidx_base"][s]) + col0 * 128) // 16
                nc.sync.dma_start(out=fi[:, :ncols * 8],
                                  in_=fidx_d[:, c0:c0 + ncols * 8])
                slots = sp.tile([128, MF * CHW], mybir.dt.bfloat16)
                nc.gpsimd.dma_gather(
                    out_ap=slots[:, :ncols * CHW].rearrange(
                        "p (m c) -> p m c", c=CHW),
                    in_ap=h_sub[:, :],
                    idxs_ap=fi[:, :ncols * 8],
                    num_idxs=ncols * 128,
                    num_idxs_reg=ncols * 128,
                    elem_size=CHW,
                    single_packet=False,
                )
                out_t = op.tile([128, TMAX * C], mybir.dt.float32)
                sl4 = slots[:, :ncols * CHW].rearrange(
                    "p (t r c) -> p t c r", r=R, c=CHW)
                nc.vector.tensor_reduce(
                    out=out_t[:, :T * C].rearrange("p (t c) -> p t c", c=C),
                    in_=sl4[:, :, 0:C, :],
                    axis=mybir.AxisListType.X,
                    op=mybir.AluOpType.add,
                )
                tb = int(meta["tile_base"][s])
                nc.sync.dma_start(
                    out=out[:, (tb + t0) * C:(tb + t0 + T) * C],
                    in_=out_t[:, :T * C])

    nc.compile()
    return nc


def assemble_output(results, perms, meta, order, n_cores):
    S = meta["S"]
    N = meta["N"]
    sub_meta = meta["sub_meta"]
    out = np.empty((N, C), np.float32)
    for c in range(n_cores):
        rows = results[c]["out"]  # [128, n_tiles_tot*C]
        for s in range(meta["NSUB"]):
            sm = sub_meta[s]
            tb = int(meta["tile_base"][s])
            arr = rows[:, tb * C:(tb + sm["n_tiles"]) * C]
            arr = arr.reshape(128, sm["n_tiles"], C).transpose(1, 0, 2).reshape(-1, C)
            dest_rows = perms[c][s]          # renumbered-space row ids
            out[order[dest_rows]] = arr[:len(dest_rows)]
    return out


N_CORES = 8
LAST_EXEC_TIME_NS = None

_CACHE = {}


def kernel(feats, weight, kernel_map):
    """Full-input entry point: shard, run on 8 NeuronCores, unshard."""
    global LAST_EXEC_TIME_NS
    import os
    from concourse import bass_utils

    feats = np.asarray(feats)
    weight = np.asarray(weight)
    kernel_map = np.asarray(kernel_map)

    in_maps, perms, meta, order = host_prep(feats, weight, kernel_map, N_CORES)
    key = (meta["GT"], meta["FT"], meta["n_tiles_tot"],
           tuple(m["HCH"] for m in meta["sub_meta"]),
           tuple(tuple(m["R_t"]) for m in meta["sub_meta"]))
    if key in _CACHE:
        nc = _CACHE[key]
    else:
        nc = build_program(N_CORES, meta)
        _CACHE[key] = nc

    trace = os.environ.get("BASS_KERNEL_TRACE", "0") == "1"
    res = bass_utils.run_bass_kernel_spmd(
        nc, in_maps, core_ids=list(range(N_CORES)), trace=trace)
    LAST_EXEC_TIME_NS = res.exec_time_ns
    return assemble_output(res.results, perms, meta, order, N_CORES)


# revision 19
# speedup vs baseline: 1.8587x; 1.8587x over previous
"""Trainium2 Bass kernel: sparse multi-head 3x3x3 voxel conv (gnn message passing).

v3: all batched DMA via the `dma_gather` extended instruction.
- Host reconstructs exact voxel linear ids from the kernel_map graph (BFS per
  connected component) and renumbers points spatially -> neighbor rank
  distance <= ~1700, so int16 gather windows work.
- Per core: 17 subpieces of 6000 dests. Per sub: one 32768-row feat window,
  k-grouped gather chunks (3 dma_gather calls of 8192 rows), per-chunk
  transpose+matmul -> H rows (256B, chunk-major, per-partition contiguous) in
  a DRAM scratch block of <=32768 rows; fold gathers slots per count-sorted
  dest tile via dma_gather from the sub's H block and reduces on DVE (f32).
"""

import sys
from contextlib import ExitStack

for p in ("/opt/trn_rl_repo", "/root/.axon_site/_ro/trn_rl_repo"):
    if p not in sys.path:
        sys.path.insert(0, p)

import numpy as np
import ml_dtypes

import concourse.tile as tile
from concourse import bass, bacc, mybir
from concourse.masks import make_identity

BF16 = ml_dtypes.bfloat16
C = 64
CHW = 128        # padded table/H row channels (256B rows)
NH = 4
CHH = 16
KVOL = 27
SIDE = 200
SUB = 6000       # dests per subpiece
MARG = 6000      # core table margin (max neighbor rank distance ~1700)
WIN = 32768
QG = 64          # chunks per gather call == H-group size
MF = 64          # max fold slot-columns per call
TMAX = 16


def cdiv(a, b):
    return (a + b - 1) // b


def spatial_order(kernel_map):
    """Exact relative voxel linear ids from the 27-neighborhood graph."""
    from scipy.sparse import coo_matrix
    from scipy.sparse.csgraph import connected_components
    N = kernel_map.shape[1]
    deltas = np.array([dx * SIDE * SIDE + dy * SIDE + dz
                       for dx in (-1, 0, 1) for dy in (-1, 0, 1) for dz in (-1, 0, 1)],
                      dtype=np.int64)
    rows = []
    cols = []
    for k in range(KVOL):
        if k == 13:
            continue
        m = kernel_map[k] >= 0
        rows.append(np.nonzero(m)[0])
        cols.append(kernel_map[k][m])
    g = coo_matrix((np.ones(sum(len(r) for r in rows), np.int8),
                    (np.concatenate(rows), np.concatenate(cols))), shape=(N, N))
    ncomp, labels = connected_components(g, directed=False)
    lin = np.zeros(N, np.int64)
    known = np.zeros(N, bool)
    _, seed_idx = np.unique(labels, return_index=True)
    lin[seed_idx] = labels[seed_idx].astype(np.int64) * (1 << 24)
    known[seed_idx] = True
    frontier = seed_idx
    while len(frontier):
        new = []
        for k in range(KVOL):
            if k == 13:
                continue
            nbr = kernel_map[k][frontier]
            ok = nbr >= 0
            nbr_v = nbr[ok]
            unk = ~known[nbr_v]
            tgt = nbr_v[unk]
            if len(tgt) == 0:
                continue
            lin[tgt] = lin[frontier[ok][unk]] + deltas[k]
            known[tgt] = True
            new.append(tgt)
        frontier = np.unique(np.concatenate(new)) if new else np.array([], np.int64)
    assert known.all()
    return np.argsort(lin, kind="stable")


def pack_idx(idx):
    """[n] int -> [128, n/16] int16: j at (j%16, j//16), replicated to 8 q7 cores."""
    n = len(idx)
    assert n % 16 == 0
    t = np.zeros((16, n // 16), np.int16)
    t[np.arange(n) % 16, np.arange(n) // 16] = idx.astype(np.int16)
    return np.tile(t, (8, 1))


def host_prep(feats, weight, kernel_map, n_cores):
    feats = np.asarray(feats)
    weight = np.asarray(weight)
    kernel_map = np.asarray(kernel_map)
    N = kernel_map.shape[1]
    S = N // n_cores
    NSUB = cdiv(S, SUB)
    CORE_ROWS = (NSUB - 1) * SUB + WIN

    order = spatial_order(kernel_map)
    inv = np.empty(N, np.int64)
    inv[order] = np.arange(N)
    feats_r = feats[order]
    kmr = np.where(kernel_map[:, order] >= 0,
                   inv[np.maximum(kernel_map[:, order], 0)], -1).astype(np.int64)

    w_sb = np.zeros((128, KVOL * C), dtype=BF16)
    for k in range(KVOL):
        blk = np.zeros((C, C), np.float32)
        for h in range(NH):
            blk[h * CHH:(h + 1) * CHH, h * CHH:(h + 1) * CHH] = weight[k, h]
        w_sb[:C, k * C:(k + 1) * C] = blk.astype(BF16)
        w_sb[C:, k * C:(k + 1) * C] = w_sb[:C, k * C:(k + 1) * C]

    # shared structure: per (sub, k) chunk counts = max over cores
    sub_lens = [min(SUB, S - s * SUB) for s in range(NSUB)]
    # per core, per sub, per k: (local dest idx within sub, local src row in core table)
    core_sub_runs = [[None] * NSUB for _ in range(n_cores)]
    for c in range(n_cores):
        tab_lo = c * S - MARG
        for s in range(NSUB):
            lo = c * S + s * SUB
            hi = lo + sub_lens[s]
            runs = []
            for k in range(KVOL):
                col = kmr[k, lo:hi]
                m = col >= 0
                d_loc = np.nonzero(m)[0]
                src_loc = col[m] - tab_lo
                runs.append((d_loc, src_loc))
            core_sub_runs[c][s] = runs

    n_chunks = np.zeros((NSUB, KVOL), np.int64)
    for s in range(NSUB):
        for k in range(KVOL):
            n_chunks[s][k] = max(cdiv(len(core_sub_runs[c][s][k][0]), 128)
                                 for c in range(n_cores))

    # per sub: chunk layout (k-major), zero chunk; uniform HCH across subs
    sub_meta = []
    for s in range(NSUB):
        chunk_k = []
        chunk_start_k = []
        for k in range(KVOL):
            chunk_start_k.append(len(chunk_k))
            chunk_k.extend([k] * int(n_chunks[s][k]))
        HCH_REAL = len(chunk_k)
        sub_meta.append(dict(chunk_k=chunk_k, chunk_start_k=chunk_start_k,
                             HCH_REAL=HCH_REAL, ZC=HCH_REAL))
    HCH = cdiv(max(m["HCH_REAL"] for m in sub_meta) + 1, QG) * QG
    assert HCH <= 256, HCH
    for m in sub_meta:
        m["HCH"] = HCH

    # fold structure per sub: count-sorted tiles, R_t = max over cores
    for s in range(NSUB):
        L = sub_lens[s]
        n_tiles = cdiv(L, 128)
        counts_sorted = np.zeros((n_cores, L), np.int64)
        for c in range(n_cores):
            counts = np.zeros(L, np.int64)
            for k in range(KVOL):
                counts[core_sub_runs[c][s][k][0]] += 1
            counts_sorted[c] = -np.sort(-counts)
        R_t = [max(1, int(counts_sorted[:, t * 128].max())) for t in range(n_tiles)]
        assert R_t[0] <= MF
        col_base = np.concatenate([[0], np.cumsum(R_t)]).astype(np.int64)
        calls = []
        t = 0
        while t < n_tiles:
            R = R_t[t]
            tb = t
            while tb < n_tiles and R_t[tb] == R:
                tb += 1
            T = max(1, min(TMAX, MF // R))
            while t < tb:
                Tc = min(T, tb - t)
                calls.append((t, Tc, R, int(col_base[t])))
                t += Tc
        sub_meta[s].update(n_tiles=n_tiles, R_t=R_t, col_base=col_base,
                           NR=int(col_base[-1]), calls=calls, L=L)

    tile_base = np.concatenate([[0], np.cumsum([m["n_tiles"] for m in sub_meta])])
    gidx_base = np.concatenate([[0], np.cumsum([m["HCH"] * 128 for m in sub_meta])])
    fidx_base = np.concatenate([[0], np.cumsum([m["NR"] * 128 for m in sub_meta])])
    GT = int(gidx_base[-1])
    FT = int(fidx_base[-1])
    n_tiles_tot = int(tile_base[-1])

    meta = dict(N=N, S=S, NSUB=NSUB, CORE_ROWS=CORE_ROWS, sub_lens=sub_lens,
                sub_meta=sub_meta, tile_base=tile_base, gidx_base=gidx_base,
                fidx_base=fidx_base, GT=GT, FT=FT, n_tiles_tot=n_tiles_tot)

    in_maps = []
    perms = []   # per core: global out row (in renumbered space) per (sub tile rank)
    for c in range(n_cores):
        tab_lo = c * S - MARG
        table = np.zeros((CORE_ROWS, CHW), dtype=BF16)
        glo = max(0, tab_lo)
        ghi = min(N, tab_lo + CORE_ROWS)
        table[glo - tab_lo:ghi - tab_lo, :C] = feats_r[glo:ghi].astype(BF16)

        gidx = np.zeros(GT, np.int64)
        fidx = np.zeros(FT, np.int64)
        core_perm = []
        for s in range(NSUB):
            sm = sub_meta[s]
            HCH = sm["HCH"]
            base_s = s * SUB
            runs = core_sub_runs[c][s]
            # gather idxs + slot map: contribution i of (k) -> slot (col, p)
            g_sub = np.zeros(HCH * 128, np.int64)   # idx j = col*128 + p
            slot_of = {}
            all_d = []
            all_slot = []
            for k in range(KVOL):
                d_loc, src_loc = runs[k]
                Lk = len(d_loc)
                if Lk:
                    j = np.arange(Lk)
                    cols = sm["chunk_start_k"][k] + j // 128
                    ps = j % 128
                    rel = src_loc - base_s
                    assert rel.min() >= 0 and rel.max() < WIN, (c, s, k, rel.min(), rel.max())
                    g_sub[cols * 128 + ps] = rel
                    all_d.append(d_loc)
                    all_slot.append(ps * HCH + cols)   # h row id
            gidx[gidx_base[s]:gidx_base[s + 1]] = g_sub

            # fold: count-sort dests within sub (per core), fill fold idxs
            L = sm["L"]
            counts = np.zeros(L, np.int64)
            ad = np.concatenate(all_d)
            aslot = np.concatenate(all_slot)
            np.add.at(counts, ad, 1)
            order_sub = np.argsort(-counts, kind="stable")
            rank = np.empty(L, np.int64)
            rank[order_sub] = np.arange(L)
            pr = rank[ad]
            o2 = np.argsort(pr, kind="stable")
            sr = pr[o2]
            sh = aslot[o2]
            grp_start = np.searchsorted(sr, np.arange(L))
            r_idx = np.arange(len(sr)) - grp_start[sr]
            t_of = sr // 128
            p_of = sr % 128
            R_arr = np.array(sm["R_t"])
            assert (r_idx < R_arr[t_of]).all()
            col = sm["col_base"][t_of] + r_idx
            f_sub = np.empty(sm["NR"] * 128, np.int64)
            # pads -> zero chunk row of same partition: p*HCH + ZC
            pcol = np.arange(sm["NR"] * 128)
            f_sub[:] = (pcol % 128) * HCH + sm["ZC"]
            f_sub[col * 128 + p_of] = sh
            fidx[fidx_base[s]:fidx_base[s + 1]] = f_sub
            core_perm.append(c * S + base_s + order_sub)

        in_maps.append({
            "table": table,
            "w_sb": w_sb,
            "gidx": pack_idx(gidx),
            "fidx": pack_idx(fidx),
        })
        perms.append(core_perm)

    return in_maps, perms, meta, order


def build_program(n_cores, meta):
    import os
    KSUBS = int(os.environ.get("KSUBS", "0")) or None      # limit #subs
    KNOFOLD = os.environ.get("KNOFOLD", "0") == "1"        # skip fold phase
    KNOGATH = os.environ.get("KNOGATH", "0") == "1"        # skip gather+compute
    NSUB = meta["NSUB"]
    sub_meta = meta["sub_meta"]
    CORE_ROWS = meta["CORE_ROWS"]
    n_tiles_tot = meta["n_tiles_tot"]

    nc = bacc.Bacc("TRN2", target_bir_lowering=False, debug=False,
                   num_devices=n_cores, num_swdge_queues=4)

    table = nc.dram_tensor("table", [CORE_ROWS, CHW], mybir.dt.bfloat16,
                           kind="ExternalInput").ap()
    w_in = nc.dram_tensor("w_sb", [128, KVOL * C], mybir.dt.bfloat16,
                          kind="ExternalInput").ap()
    gidx_d = nc.dram_tensor("gidx", [128, meta["GT"] // 16], mybir.dt.int16,
                            kind="ExternalInput").ap()
    fidx_d = nc.dram_tensor("fidx", [128, meta["FT"] // 16], mybir.dt.int16,
                            kind="ExternalInput").ap()
    out = nc.dram_tensor("out", [128, n_tiles_tot * C], mybir.dt.float32,
                         kind="ExternalOutput").ap()

    with tile.TileContext(nc) as tc, ExitStack() as ctx:
        dram = ctx.enter_context(tc.tile_pool(name="dram", bufs=2, space="DRAM"))

        wpool = ctx.enter_context(tc.tile_pool(name="w", bufs=1))
        w_t = wpool.tile([128, KVOL * C], mybir.dt.bfloat16)
        nc.sync.dma_start(out=w_t[:], in_=w_in[:])
        ident = wpool.tile([128, 128], mybir.dt.bfloat16)
        make_identity(nc, ident[:])

        gip = ctx.enter_context(tc.tile_pool(name="gi", bufs=3))
        fip = ctx.enter_context(tc.tile_pool(name="fi", bufs=3))
        gp = ctx.enter_context(tc.tile_pool(name="G", bufs=2))
        xp = ctx.enter_context(tc.tile_pool(name="X", bufs=8))
        hp = ctx.enter_context(tc.tile_pool(name="H", bufs=2))
        sp = ctx.enter_context(tc.tile_pool(name="slots", bufs=2))
        op = ctx.enter_context(tc.tile_pool(name="outp", bufs=3))
        psx = ctx.enter_context(tc.tile_pool(name="psx", bufs=4, space="PSUM"))
        psh = ctx.enter_context(tc.tile_pool(name="psh", bufs=4, space="PSUM"))

        for s in range(NSUB if KSUBS is None else min(NSUB, KSUBS)):
            sm = sub_meta[s]
            HCH = sm["HCH"]
            chunk_k = sm["chunk_k"]
            HCH_REAL = sm["HCH_REAL"]
            base_s = s * SUB

            h_sub = dram.tile([128 * HCH, CHW], mybir.dt.bfloat16)

            # groups entirely past the zero chunk are never referenced
            g_last = cdiv(HCH_REAL + 1, QG)
            for q0 in ([] if KNOGATH else range(0, g_last * QG, QG)):
                # gather only real chunk columns (zero/pad cols are memset)
                nreal = max(0, min(QG, HCH_REAL - q0))
                ncg = max(nreal, 1)
                gi = gip.tile([128, QG * 8], mybir.dt.int16)
                c0 = (int(meta["gidx_base"][s]) + q0 * 128) // 16
                nc.sync.dma_start(out=gi[:, :ncg * 8],
                                  in_=gidx_d[:, c0:c0 + ncg * 8])
                gbuf = gp.tile([128, QG * CHW], mybir.dt.bfloat16)
                nc.gpsimd.dma_gather(
                    out_ap=gbuf[:, :ncg * CHW].rearrange(
                        "p (m c) -> p m c", c=CHW),
                    in_ap=table[base_s:base_s + WIN, :],
                    idxs_ap=gi[:, :ncg * 8],
                    num_idxs=ncg * 128,
                    num_idxs_reg=ncg * 128,
                    elem_size=CHW,
                    single_packet=False,
                )
                h_t = hp.tile([128, QG * CHW], mybir.dt.bfloat16)
                if q0 + QG > HCH_REAL:
                    # group contains pad/zero chunks: zero the whole tile so
                    # fold pads (and H-write reads) see defined zeros
                    nc.vector.memset(h_t[:], 0.0)
                for q in range(QG):
                    colc = q0 + q
                    if colc >= HCH_REAL:
                        continue
                    k = chunk_k[colc]
                    x_ps = psx.tile([64, 128], mybir.dt.bfloat16)
                    nc.tensor.transpose(
                        out=x_ps[:], in_=gbuf[:, q * CHW:q * CHW + C],
                        identity=ident[:])
                    x_t = xp.tile([64, 128], mybir.dt.bfloat16)
                    nc.vector.tensor_copy(out=x_t[:], in_=x_ps[:])
                    h_ps = psh.tile([128, C], mybir.dt.float32)
                    nc.tensor.matmul(
                        out=h_ps[:],
                        lhsT=x_t[:],
                        rhs=w_t[0:64, k * C:(k + 1) * C],
                        start=True, stop=True,
                    )
                    nc.scalar.activation(
                        h_t[:, q * CHW:q * CHW + C], h_ps[:],
                        mybir.ActivationFunctionType.Copy,
                    )
                # H rows: slot (col, p) -> row p*HCH + col; this group: cols [q0, q0+QG)
                nc.sync.dma_start(
                    out=h_sub[:].rearrange(
                        "(p q) c -> p q c", q=HCH)[:, q0:q0 + QG, :],
                    in_=h_t[:].rearrange("p (q c) -> p q c", c=CHW),
                )

            for (t0, T, R, col0) in ([] if KNOFOLD else sm["calls"]):
                ncols = T * R
                fi = fip.tile([128, MF * 8], mybir.dt.int16)
                c0 = (int(meta["fidx_base"][s]) + col0 * 128) // 16
                nc.sync.dma_start(out=fi[:, :ncols * 8],
                                  in_=fidx_d[:, c0:c0 + ncols * 8])
                slots = sp.tile([128, MF * CHW], mybir.dt.bfloat16)
                nc.gpsimd.dma_gather(
                    out_ap=slots[:, :ncols * CHW].rearrange(
                        "p (m c) -> p m c", c=CHW),
                    in_ap=h_sub[:, :],
                    idxs_ap=fi[:, :ncols * 8],
                    num_idxs=ncols * 128,
                    num_idxs_reg=ncols * 128,
                    elem_size=CHW,
                    single_packet=False,
                )
                out_t = op.tile([128, TMAX * C], mybir.dt.float32)
                sl4 = slots[:, :ncols * CHW].rearrange(
                    "p (t r c) -> p t c r", r=R, c=CHW)
                nc.vector.tensor_reduce(
                    out=out_t[:, :T * C].rearrange("p (t c) -> p t c", c=C),
                    in_=sl4[:, :, 0:C, :],
                    axis=mybir.AxisListType.X,
                    op=mybir.AluOpType.add,
                )
                tb = int(meta["tile_base"][s])
                nc.sync.dma_start(
                    out=out[:, (tb + t0) * C:(tb + t0 + T) * C],
                    in_=out_t[:, :T * C])

    nc.compile()
    # spread gathers across the 4 SWDGE queues (Q7 core pairs) consistently
    # with the DMASW sem lane Tile assigned: queue = lane % 4 keeps each sem
    # locked to a single queue while 4 Q7 pairs generate descriptors in parallel
    import re
    nq = 0
    for bb in nc.main_func.blocks:
        for inst in bb.instructions:
            if isinstance(inst, mybir.InstDMAGatherAnt):
                m = re.match(r"DMASW(\d+)_", inst.sync_info.on_update[0].ant_name)
                if m:
                    inst.queue_num = int(m.group(1)) % 4
                    nq += 1
    return nc


def assemble_output(results, perms, meta, order, n_cores):
    S = meta["S"]
    N = meta["N"]
    sub_meta = meta["sub_meta"]
    out = np.empty((N, C), np.float32)
    for c in range(n_cores):
        rows = results[c]["out"]  # [128, n_tiles_tot*C]
        for s in range(meta["NSUB"]):
            sm = sub_meta[s]
            tb = int(meta["tile_base"][s])
            arr = rows[:, tb * C:(tb + sm["n_tiles"]) * C]
            arr = arr.reshape(128, sm["n_tiles"], C).transpose(1, 0, 2).reshape(-1, C)
            dest_rows = perms[c][s]          # renumbered-space row ids
            out[order[dest_rows]] = arr[:len(dest_rows)]
    return out


N_CORES = 8
LAST_EXEC_TIME_NS = None

_CACHE = {}


def kernel(feats, weight, kernel_map):
    """Full-input entry point: shard, run on 8 NeuronCores, unshard."""
    global LAST_EXEC_TIME_NS
    import os
    from concourse import bass_utils

    feats = np.asarray(feats)
    weight = np.asarray(weight)
    kernel_map = np.asarray(kernel_map)

    in_maps, perms, meta, order = host_prep(feats, weight, kernel_map, N_CORES)
    key = (meta["GT"], meta["FT"], meta["n_tiles_tot"],
           tuple(m["HCH"] for m in meta["sub_meta"]),
           tuple(tuple(m["R_t"]) for m in meta["sub_meta"]))
    if key in _CACHE:
        nc = _CACHE[key]
    else:
        nc = build_program(N_CORES, meta)
        _CACHE[key] = nc

    trace = os.environ.get("BASS_KERNEL_TRACE", "0") == "1"
    res = bass_utils.run_bass_kernel_spmd(
        nc, in_maps, core_ids=list(range(N_CORES)), trace=trace)
    LAST_EXEC_TIME_NS = res.exec_time_ns
    return assemble_output(res.results, perms, meta, order, N_CORES)


# revision 20
# speedup vs baseline: 3.0501x; 1.6410x over previous
"""Trainium2 Bass kernel: sparse multi-head 3x3x3 voxel conv (gnn message passing).

v3: all batched DMA via the `dma_gather` extended instruction.
- Host reconstructs exact voxel linear ids from the kernel_map graph (BFS per
  connected component) and renumbers points spatially -> neighbor rank
  distance <= ~1700, so int16 gather windows work.
- Per core: 17 subpieces of 6000 dests. Per sub: one 32768-row feat window,
  k-grouped gather chunks (3 dma_gather calls of 8192 rows), per-chunk
  transpose+matmul -> H rows (256B, chunk-major, per-partition contiguous) in
  a DRAM scratch block of <=32768 rows; fold gathers slots per count-sorted
  dest tile via dma_gather from the sub's H block and reduces on DVE (f32).
"""

import sys
from contextlib import ExitStack

for p in ("/opt/trn_rl_repo", "/root/.axon_site/_ro/trn_rl_repo"):
    if p not in sys.path:
        sys.path.insert(0, p)

import numpy as np
import ml_dtypes

import concourse.tile as tile
from concourse import bass, bacc, mybir
from concourse.masks import make_identity

BF16 = ml_dtypes.bfloat16
C = 64
CHW = 128        # padded table/H row channels (256B rows)
NH = 4
CHH = 16
KVOL = 27
SIDE = 200
SUB = 6000       # dests per subpiece
MARG = 6000      # core table margin (max neighbor rank distance ~1700)
WIN = 32768
QG = 32          # chunks per gather call == H-group size
MF = 64          # max fold slot-columns per call
TMAX = 16


def cdiv(a, b):
    return (a + b - 1) // b


def spatial_order(kernel_map):
    """Exact relative voxel linear ids from the 27-neighborhood graph."""
    from scipy.sparse import coo_matrix
    from scipy.sparse.csgraph import connected_components
    N = kernel_map.shape[1]
    deltas = np.array([dx * SIDE * SIDE + dy * SIDE + dz
                       for dx in (-1, 0, 1) for dy in (-1, 0, 1) for dz in (-1, 0, 1)],
                      dtype=np.int64)
    rows = []
    cols = []
    for k in range(KVOL):
        if k == 13:
            continue
        m = kernel_map[k] >= 0
        rows.append(np.nonzero(m)[0])
        cols.append(kernel_map[k][m])
    g = coo_matrix((np.ones(sum(len(r) for r in rows), np.int8),
                    (np.concatenate(rows), np.concatenate(cols))), shape=(N, N))
    ncomp, labels = connected_components(g, directed=False)
    lin = np.zeros(N, np.int64)
    known = np.zeros(N, bool)
    _, seed_idx = np.unique(labels, return_index=True)
    lin[seed_idx] = labels[seed_idx].astype(np.int64) * (1 << 24)
    known[seed_idx] = True
    frontier = seed_idx
    while len(frontier):
        new = []
        for k in range(KVOL):
            if k == 13:
                continue
            nbr = kernel_map[k][frontier]
            ok = nbr >= 0
            nbr_v = nbr[ok]
            unk = ~known[nbr_v]
            tgt = nbr_v[unk]
            if len(tgt) == 0:
                continue
            lin[tgt] = lin[frontier[ok][unk]] + deltas[k]
            known[tgt] = True
            new.append(tgt)
        frontier = np.unique(np.concatenate(new)) if new else np.array([], np.int64)
    assert known.all()
    return np.argsort(lin, kind="stable")


def pack_idx(idx):
    """[n] int -> [128, n/16] int16: j at (j%16, j//16), replicated to 8 q7 cores."""
    n = len(idx)
    assert n % 16 == 0
    t = np.zeros((16, n // 16), np.int16)
    t[np.arange(n) % 16, np.arange(n) // 16] = idx.astype(np.int16)
    return np.tile(t, (8, 1))


def host_prep(feats, weight, kernel_map, n_cores):
    feats = np.asarray(feats)
    weight = np.asarray(weight)
    kernel_map = np.asarray(kernel_map)
    N = kernel_map.shape[1]
    S = N // n_cores
    NSUB = cdiv(S, SUB)
    CORE_ROWS = (NSUB - 1) * SUB + WIN

    order = spatial_order(kernel_map)
    inv = np.empty(N, np.int64)
    inv[order] = np.arange(N)
    feats_r = feats[order]
    kmr = np.where(kernel_map[:, order] >= 0,
                   inv[np.maximum(kernel_map[:, order], 0)], -1).astype(np.int64)

    w_sb = np.zeros((128, KVOL * C), dtype=BF16)
    for k in range(KVOL):
        blk = np.zeros((C, C), np.float32)
        for h in range(NH):
            blk[h * CHH:(h + 1) * CHH, h * CHH:(h + 1) * CHH] = weight[k, h]
        w_sb[:C, k * C:(k + 1) * C] = blk.astype(BF16)
        w_sb[C:, k * C:(k + 1) * C] = w_sb[:C, k * C:(k + 1) * C]

    # shared structure: per (sub, k) chunk counts = max over cores
    sub_lens = [min(SUB, S - s * SUB) for s in range(NSUB)]
    # per core, per sub, per k: (local dest idx within sub, local src row in core table)
    core_sub_runs = [[None] * NSUB for _ in range(n_cores)]
    for c in range(n_cores):
        tab_lo = c * S - MARG
        for s in range(NSUB):
            lo = c * S + s * SUB
            hi = lo + sub_lens[s]
            runs = []
            for k in range(KVOL):
                col = kmr[k, lo:hi]
                m = col >= 0
                d_loc = np.nonzero(m)[0]
                src_loc = col[m] - tab_lo
                runs.append((d_loc, src_loc))
            core_sub_runs[c][s] = runs

    n_chunks = np.zeros((NSUB, KVOL), np.int64)
    for s in range(NSUB):
        for k in range(KVOL):
            n_chunks[s][k] = max(cdiv(len(core_sub_runs[c][s][k][0]), 128)
                                 for c in range(n_cores))

    # per sub: chunk layout (k-major), zero chunk; uniform HCH across subs
    sub_meta = []
    for s in range(NSUB):
        chunk_k = []
        chunk_start_k = []
        for k in range(KVOL):
            chunk_start_k.append(len(chunk_k))
            chunk_k.extend([k] * int(n_chunks[s][k]))
        HCH_REAL = len(chunk_k)
        sub_meta.append(dict(chunk_k=chunk_k, chunk_start_k=chunk_start_k,
                             HCH_REAL=HCH_REAL, ZC=HCH_REAL))
    HCH = cdiv(max(m["HCH_REAL"] for m in sub_meta) + 1, QG) * QG
    assert HCH <= 256, HCH
    for m in sub_meta:
        m["HCH"] = HCH

    # fold structure per sub: count-sorted tiles, R_t = max over cores
    for s in range(NSUB):
        L = sub_lens[s]
        n_tiles = cdiv(L, 128)
        counts_sorted = np.zeros((n_cores, L), np.int64)
        for c in range(n_cores):
            counts = np.zeros(L, np.int64)
            for k in range(KVOL):
                counts[core_sub_runs[c][s][k][0]] += 1
            counts_sorted[c] = -np.sort(-counts)
        R_t = [max(1, int(counts_sorted[:, t * 128].max())) for t in range(n_tiles)]
        assert R_t[0] <= MF
        col_base = np.concatenate([[0], np.cumsum(R_t)]).astype(np.int64)
        calls = []
        t = 0
        while t < n_tiles:
            R = R_t[t]
            tb = t
            while tb < n_tiles and R_t[tb] == R:
                tb += 1
            T = max(1, min(TMAX, MF // R))
            while t < tb:
                Tc = min(T, tb - t)
                calls.append((t, Tc, R, int(col_base[t])))
                t += Tc
        sub_meta[s].update(n_tiles=n_tiles, R_t=R_t, col_base=col_base,
                           NR=int(col_base[-1]), calls=calls, L=L)

    tile_base = np.concatenate([[0], np.cumsum([m["n_tiles"] for m in sub_meta])])
    gidx_base = np.concatenate([[0], np.cumsum([m["HCH"] * 128 for m in sub_meta])])
    fidx_base = np.concatenate([[0], np.cumsum([m["NR"] * 128 for m in sub_meta])])
    GT = int(gidx_base[-1])
    FT = int(fidx_base[-1])
    n_tiles_tot = int(tile_base[-1])

    meta = dict(N=N, S=S, NSUB=NSUB, CORE_ROWS=CORE_ROWS, sub_lens=sub_lens,
                sub_meta=sub_meta, tile_base=tile_base, gidx_base=gidx_base,
                fidx_base=fidx_base, GT=GT, FT=FT, n_tiles_tot=n_tiles_tot)

    in_maps = []
    perms = []   # per core: global out row (in renumbered space) per (sub tile rank)
    for c in range(n_cores):
        tab_lo = c * S - MARG
        table = np.zeros((CORE_ROWS, CHW), dtype=BF16)
        glo = max(0, tab_lo)
        ghi = min(N, tab_lo + CORE_ROWS)
        table[glo - tab_lo:ghi - tab_lo, :C] = feats_r[glo:ghi].astype(BF16)

        gidx = np.zeros(GT, np.int64)
        fidx = np.zeros(FT, np.int64)
        core_perm = []
        for s in range(NSUB):
            sm = sub_meta[s]
            HCH = sm["HCH"]
            base_s = s * SUB
            runs = core_sub_runs[c][s]
            # gather idxs + slot map: contribution i of (k) -> slot (col, p)
            g_sub = np.zeros(HCH * 128, np.int64)   # idx j = col*128 + p
            slot_of = {}
            all_d = []
            all_slot = []
            for k in range(KVOL):
                d_loc, src_loc = runs[k]
                Lk = len(d_loc)
                if Lk:
                    j = np.arange(Lk)
                    cols = sm["chunk_start_k"][k] + j // 128
                    ps = j % 128
                    rel = src_loc - base_s
                    assert rel.min() >= 0 and rel.max() < WIN, (c, s, k, rel.min(), rel.max())
                    g_sub[cols * 128 + ps] = rel
                    all_d.append(d_loc)
                    all_slot.append(ps * HCH + cols)   # h row id
            gidx[gidx_base[s]:gidx_base[s + 1]] = g_sub

            # fold: count-sort dests within sub (per core), fill fold idxs
            L = sm["L"]
            counts = np.zeros(L, np.int64)
            ad = np.concatenate(all_d)
            aslot = np.concatenate(all_slot)
            np.add.at(counts, ad, 1)
            order_sub = np.argsort(-counts, kind="stable")
            rank = np.empty(L, np.int64)
            rank[order_sub] = np.arange(L)
            pr = rank[ad]
            o2 = np.argsort(pr, kind="stable")
            sr = pr[o2]
            sh = aslot[o2]
            grp_start = np.searchsorted(sr, np.arange(L))
            r_idx = np.arange(len(sr)) - grp_start[sr]
            t_of = sr // 128
            p_of = sr % 128
            R_arr = np.array(sm["R_t"])
            assert (r_idx < R_arr[t_of]).all()
            col = sm["col_base"][t_of] + r_idx
            f_sub = np.empty(sm["NR"] * 128, np.int64)
            # pads -> zero chunk row of same partition: p*HCH + ZC
            pcol = np.arange(sm["NR"] * 128)
            f_sub[:] = (pcol % 128) * HCH + sm["ZC"]
            f_sub[col * 128 + p_of] = sh
            fidx[fidx_base[s]:fidx_base[s + 1]] = f_sub
            core_perm.append(c * S + base_s + order_sub)

        in_maps.append({
            "table": table,
            "w_sb": w_sb,
            "gidx": pack_idx(gidx),
            "fidx": pack_idx(fidx),
        })
        perms.append(core_perm)

    return in_maps, perms, meta, order


def build_program(n_cores, meta):
    import os
    KSUBS = int(os.environ.get("KSUBS", "0")) or None      # limit #subs
    KNOFOLD = os.environ.get("KNOFOLD", "0") == "1"        # skip fold phase
    KNOGATH = os.environ.get("KNOGATH", "0") == "1"        # skip gather+compute
    NSUB = meta["NSUB"]
    sub_meta = meta["sub_meta"]
    CORE_ROWS = meta["CORE_ROWS"]
    n_tiles_tot = meta["n_tiles_tot"]

    nc = bacc.Bacc("TRN2", target_bir_lowering=False, debug=False,
                   num_devices=n_cores, num_swdge_queues=4)

    table = nc.dram_tensor("table", [CORE_ROWS, CHW], mybir.dt.bfloat16,
                           kind="ExternalInput").ap()
    w_in = nc.dram_tensor("w_sb", [128, KVOL * C], mybir.dt.bfloat16,
                          kind="ExternalInput").ap()
    gidx_d = nc.dram_tensor("gidx", [128, meta["GT"] // 16], mybir.dt.int16,
                            kind="ExternalInput").ap()
    fidx_d = nc.dram_tensor("fidx", [128, meta["FT"] // 16], mybir.dt.int16,
                            kind="ExternalInput").ap()
    out = nc.dram_tensor("out", [128, n_tiles_tot * C], mybir.dt.float32,
                         kind="ExternalOutput").ap()

    with tile.TileContext(nc) as tc, ExitStack() as ctx:
        dram = ctx.enter_context(tc.tile_pool(name="dram", bufs=2, space="DRAM"))

        wpool = ctx.enter_context(tc.tile_pool(name="w", bufs=1))
        w_t = wpool.tile([128, KVOL * C], mybir.dt.bfloat16)
        nc.sync.dma_start(out=w_t[:], in_=w_in[:])
        ident = wpool.tile([128, 128], mybir.dt.bfloat16)
        make_identity(nc, ident[:])

        gip = ctx.enter_context(tc.tile_pool(name="gi", bufs=4))
        fip = ctx.enter_context(tc.tile_pool(name="fi", bufs=4))
        gp = ctx.enter_context(tc.tile_pool(name="G", bufs=5))
        xp = ctx.enter_context(tc.tile_pool(name="X", bufs=8))
        hp = ctx.enter_context(tc.tile_pool(name="H", bufs=3))
        sp = ctx.enter_context(tc.tile_pool(name="slots", bufs=3))
        op = ctx.enter_context(tc.tile_pool(name="outp", bufs=3))
        psx = ctx.enter_context(tc.tile_pool(name="psx", bufs=4, space="PSUM"))
        psh = ctx.enter_context(tc.tile_pool(name="psh", bufs=4, space="PSUM"))

        for s in range(NSUB if KSUBS is None else min(NSUB, KSUBS)):
            sm = sub_meta[s]
            HCH = sm["HCH"]
            chunk_k = sm["chunk_k"]
            HCH_REAL = sm["HCH_REAL"]
            base_s = s * SUB

            h_sub = dram.tile([128 * HCH, CHW], mybir.dt.bfloat16)

            # groups entirely past the zero chunk are never referenced
            g_last = cdiv(HCH_REAL + 1, QG)
            for q0 in ([] if KNOGATH else range(0, g_last * QG, QG)):
                # gather only real chunk columns (zero/pad cols are memset)
                nreal = max(0, min(QG, HCH_REAL - q0))
                ncg = max(nreal, 1)
                gi = gip.tile([128, QG * 8], mybir.dt.int16)
                c0 = (int(meta["gidx_base"][s]) + q0 * 128) // 16
                nc.sync.dma_start(out=gi[:, :ncg * 8],
                                  in_=gidx_d[:, c0:c0 + ncg * 8])
                gbuf = gp.tile([128, QG * CHW], mybir.dt.bfloat16)
                nc.gpsimd.dma_gather(
                    out_ap=gbuf[:, :ncg * CHW].rearrange(
                        "p (m c) -> p m c", c=CHW),
                    in_ap=table[base_s:base_s + WIN, :],
                    idxs_ap=gi[:, :ncg * 8],
                    num_idxs=ncg * 128,
                    num_idxs_reg=ncg * 128,
                    elem_size=CHW,
                    single_packet=False,
                )
                h_t = hp.tile([128, QG * CHW], mybir.dt.bfloat16)
                if q0 + QG > HCH_REAL:
                    # group contains pad/zero chunks: zero the whole tile so
                    # fold pads (and H-write reads) see defined zeros
                    nc.vector.memset(h_t[:], 0.0)
                for q in range(QG):
                    colc = q0 + q
                    if colc >= HCH_REAL:
                        continue
                    k = chunk_k[colc]
                    x_ps = psx.tile([64, 128], mybir.dt.bfloat16)
                    nc.tensor.transpose(
                        out=x_ps[:], in_=gbuf[:, q * CHW:q * CHW + C],
                        identity=ident[:])
                    x_t = xp.tile([64, 128], mybir.dt.bfloat16)
                    nc.vector.tensor_copy(out=x_t[:], in_=x_ps[:])
                    h_ps = psh.tile([128, C], mybir.dt.float32)
                    nc.tensor.matmul(
                        out=h_ps[:],
                        lhsT=x_t[:],
                        rhs=w_t[0:64, k * C:(k + 1) * C],
                        start=True, stop=True,
                    )
                    nc.scalar.activation(
                        h_t[:, q * CHW:q * CHW + C], h_ps[:],
                        mybir.ActivationFunctionType.Copy,
                    )
                # H rows: slot (col, p) -> row p*HCH + col; this group: cols [q0, q0+QG)
                nc.sync.dma_start(
                    out=h_sub[:].rearrange(
                        "(p q) c -> p q c", q=HCH)[:, q0:q0 + QG, :],
                    in_=h_t[:].rearrange("p (q c) -> p q c", c=CHW),
                )

            for (t0, T, R, col0) in ([] if KNOFOLD else sm["calls"]):
                ncols = T * R
                fi = fip.tile([128, MF * 8], mybir.dt.int16)
                c0 = (int(meta["fidx_base"][s]) + col0 * 128) // 16
                nc.sync.dma_start(out=fi[:, :ncols * 8],
                                  in_=fidx_d[:, c0:c0 + ncols * 8])
                slots = sp.tile([128, MF * CHW], mybir.dt.bfloat16)
                nc.gpsimd.dma_gather(
                    out_ap=slots[:, :ncols * CHW].rearrange(
                        "p (m c) -> p m c", c=CHW),
                    in_ap=h_sub[:, :],
                    idxs_ap=fi[:, :ncols * 8],
                    num_idxs=ncols * 128,
                    num_idxs_reg=ncols * 128,
                    elem_size=CHW,
                    single_packet=False,
                )
                out_t = op.tile([128, TMAX * C], mybir.dt.float32)
                sl4 = slots[:, :ncols * CHW].rearrange(
                    "p (t r c) -> p t c r", r=R, c=CHW)
                nc.vector.tensor_reduce(
                    out=out_t[:, :T * C].rearrange("p (t c) -> p t c", c=C),
                    in_=sl4[:, :, 0:C, :],
                    axis=mybir.AxisListType.X,
                    op=mybir.AluOpType.add,
                )
                tb = int(meta["tile_base"][s])
                nc.sync.dma_start(
                    out=out[:, (tb + t0) * C:(tb + t0 + T) * C],
                    in_=out_t[:, :T * C])

    nc.compile()
    # spread gathers across the 4 SWDGE queues (Q7 core pairs) consistently
    # with the DMASW sem lane Tile assigned: queue = lane % 4 keeps each sem
    # locked to a single queue while 4 Q7 pairs generate descriptors in parallel
    import re
    nq = 0
    for bb in nc.main_func.blocks:
        for inst in bb.instructions:
            if isinstance(inst, mybir.InstDMAGatherAnt):
                m = re.match(r"DMASW(\d+)_", inst.sync_info.on_update[0].ant_name)
                if m:
                    inst.queue_num = int(m.group(1)) % 4
                    nq += 1
    return nc


def assemble_output(results, perms, meta, order, n_cores):
    S = meta["S"]
    N = meta["N"]
    sub_meta = meta["sub_meta"]
    out = np.empty((N, C), np.float32)
    for c in range(n_cores):
        rows = results[c]["out"]  # [128, n_tiles_tot*C]
        for s in range(meta["NSUB"]):
            sm = sub_meta[s]
            tb = int(meta["tile_base"][s])
            arr = rows[:, tb * C:(tb + sm["n_tiles"]) * C]
            arr = arr.reshape(128, sm["n_tiles"], C).transpose(1, 0, 2).reshape(-1, C)
            dest_rows = perms[c][s]          # renumbered-space row ids
            out[order[dest_rows]] = arr[:len(dest_rows)]
    return out


N_CORES = 8
LAST_EXEC_TIME_NS = None

_CACHE = {}


def kernel(feats, weight, kernel_map):
    """Full-input entry point: shard, run on 8 NeuronCores, unshard."""
    global LAST_EXEC_TIME_NS
    import os
    from concourse import bass_utils

    feats = np.asarray(feats)
    weight = np.asarray(weight)
    kernel_map = np.asarray(kernel_map)

    in_maps, perms, meta, order = host_prep(feats, weight, kernel_map, N_CORES)
    key = (meta["GT"], meta["FT"], meta["n_tiles_tot"],
           tuple(m["HCH"] for m in meta["sub_meta"]),
           tuple(tuple(m["R_t"]) for m in meta["sub_meta"]))
    if key in _CACHE:
        nc = _CACHE[key]
    else:
        nc = build_program(N_CORES, meta)
        _CACHE[key] = nc

    trace = os.environ.get("BASS_KERNEL_TRACE", "0") == "1"
    res = bass_utils.run_bass_kernel_spmd(
        nc, in_maps, core_ids=list(range(N_CORES)), trace=trace)
    LAST_EXEC_TIME_NS = res.exec_time_ns
    return assemble_output(res.results, perms, meta, order, N_CORES)


# revision 21
# speedup vs baseline: 3.1959x; 1.0478x over previous
"""Trainium2 Bass kernel: sparse multi-head 3x3x3 voxel conv (gnn message passing).

v3: all batched DMA via the `dma_gather` extended instruction.
- Host reconstructs exact voxel linear ids from the kernel_map graph (BFS per
  connected component) and renumbers points spatially -> neighbor rank
  distance <= ~1700, so int16 gather windows work.
- Per core: 17 subpieces of 6000 dests. Per sub: one 32768-row feat window,
  k-grouped gather chunks (3 dma_gather calls of 8192 rows), per-chunk
  transpose+matmul -> H rows (256B, chunk-major, per-partition contiguous) in
  a DRAM scratch block of <=32768 rows; fold gathers slots per count-sorted
  dest tile via dma_gather from the sub's H block and reduces on DVE (f32).
"""

import sys
from contextlib import ExitStack

for p in ("/opt/trn_rl_repo", "/root/.axon_site/_ro/trn_rl_repo"):
    if p not in sys.path:
        sys.path.insert(0, p)

import numpy as np
import ml_dtypes

import concourse.tile as tile
from concourse import bass, bacc, mybir
from concourse.masks import make_identity

BF16 = ml_dtypes.bfloat16
C = 64
CHW = 128        # padded table/H row channels (256B rows)
NH = 4
CHH = 16
KVOL = 27
SIDE = 200
SUB = 6000       # dests per subpiece
MARG = 6000      # core table margin (max neighbor rank distance ~1700)
WIN = 32768
QG = 32          # chunks per gather call == H-group size
MF = 32          # max fold slot-columns per call
TMAX = 16


def cdiv(a, b):
    return (a + b - 1) // b


def spatial_order(kernel_map):
    """Exact relative voxel linear ids from the 27-neighborhood graph."""
    from scipy.sparse import coo_matrix
    from scipy.sparse.csgraph import connected_components
    N = kernel_map.shape[1]
    deltas = np.array([dx * SIDE * SIDE + dy * SIDE + dz
                       for dx in (-1, 0, 1) for dy in (-1, 0, 1) for dz in (-1, 0, 1)],
                      dtype=np.int64)
    rows = []
    cols = []
    for k in range(KVOL):
        if k == 13:
            continue
        m = kernel_map[k] >= 0
        rows.append(np.nonzero(m)[0])
        cols.append(kernel_map[k][m])
    g = coo_matrix((np.ones(sum(len(r) for r in rows), np.int8),
                    (np.concatenate(rows), np.concatenate(cols))), shape=(N, N))
    ncomp, labels = connected_components(g, directed=False)
    lin = np.zeros(N, np.int64)
    known = np.zeros(N, bool)
    _, seed_idx = np.unique(labels, return_index=True)
    lin[seed_idx] = labels[seed_idx].astype(np.int64) * (1 << 24)
    known[seed_idx] = True
    frontier = seed_idx
    while len(frontier):
        new = []
        for k in range(KVOL):
            if k == 13:
                continue
            nbr = kernel_map[k][frontier]
            ok = nbr >= 0
            nbr_v = nbr[ok]
            unk = ~known[nbr_v]
            tgt = nbr_v[unk]
            if len(tgt) == 0:
                continue
            lin[tgt] = lin[frontier[ok][unk]] + deltas[k]
            known[tgt] = True
            new.append(tgt)
        frontier = np.unique(np.concatenate(new)) if new else np.array([], np.int64)
    assert known.all()
    return np.argsort(lin, kind="stable")


def pack_idx(idx):
    """[n] int -> [128, n/16] int16: j at (j%16, j//16), replicated to 8 q7 cores."""
    n = len(idx)
    assert n % 16 == 0
    t = np.zeros((16, n // 16), np.int16)
    t[np.arange(n) % 16, np.arange(n) // 16] = idx.astype(np.int16)
    return np.tile(t, (8, 1))


def host_prep(feats, weight, kernel_map, n_cores):
    feats = np.asarray(feats)
    weight = np.asarray(weight)
    kernel_map = np.asarray(kernel_map)
    N = kernel_map.shape[1]
    S = N // n_cores
    NSUB = cdiv(S, SUB)
    CORE_ROWS = (NSUB - 1) * SUB + WIN

    order = spatial_order(kernel_map)
    inv = np.empty(N, np.int64)
    inv[order] = np.arange(N)
    feats_r = feats[order]
    kmr = np.where(kernel_map[:, order] >= 0,
                   inv[np.maximum(kernel_map[:, order], 0)], -1).astype(np.int64)

    w_sb = np.zeros((128, KVOL * C), dtype=BF16)
    for k in range(KVOL):
        blk = np.zeros((C, C), np.float32)
        for h in range(NH):
            blk[h * CHH:(h + 1) * CHH, h * CHH:(h + 1) * CHH] = weight[k, h]
        w_sb[:C, k * C:(k + 1) * C] = blk.astype(BF16)
        w_sb[C:, k * C:(k + 1) * C] = w_sb[:C, k * C:(k + 1) * C]

    # shared structure: per (sub, k) chunk counts = max over cores
    sub_lens = [min(SUB, S - s * SUB) for s in range(NSUB)]
    # per core, per sub, per k: (local dest idx within sub, local src row in core table)
    core_sub_runs = [[None] * NSUB for _ in range(n_cores)]
    for c in range(n_cores):
        tab_lo = c * S - MARG
        for s in range(NSUB):
            lo = c * S + s * SUB
            hi = lo + sub_lens[s]
            runs = []
            for k in range(KVOL):
                col = kmr[k, lo:hi]
                m = col >= 0
                d_loc = np.nonzero(m)[0]
                src_loc = col[m] - tab_lo
                runs.append((d_loc, src_loc))
            core_sub_runs[c][s] = runs

    n_chunks = np.zeros((NSUB, KVOL), np.int64)
    for s in range(NSUB):
        for k in range(KVOL):
            n_chunks[s][k] = max(cdiv(len(core_sub_runs[c][s][k][0]), 128)
                                 for c in range(n_cores))

    # per sub: chunk layout (k-major), zero chunk; uniform HCH across subs
    sub_meta = []
    for s in range(NSUB):
        chunk_k = []
        chunk_start_k = []
        for k in range(KVOL):
            chunk_start_k.append(len(chunk_k))
            chunk_k.extend([k] * int(n_chunks[s][k]))
        HCH_REAL = len(chunk_k)
        sub_meta.append(dict(chunk_k=chunk_k, chunk_start_k=chunk_start_k,
                             HCH_REAL=HCH_REAL, ZC=HCH_REAL))
    HCH = cdiv(max(m["HCH_REAL"] for m in sub_meta) + 1, QG) * QG
    assert HCH <= 256, HCH
    for m in sub_meta:
        m["HCH"] = HCH

    # fold structure per sub: count-sorted tiles, R_t = max over cores
    for s in range(NSUB):
        L = sub_lens[s]
        n_tiles = cdiv(L, 128)
        counts_sorted = np.zeros((n_cores, L), np.int64)
        for c in range(n_cores):
            counts = np.zeros(L, np.int64)
            for k in range(KVOL):
                counts[core_sub_runs[c][s][k][0]] += 1
            counts_sorted[c] = -np.sort(-counts)
        R_t = [max(1, int(counts_sorted[:, t * 128].max())) for t in range(n_tiles)]
        assert R_t[0] <= MF
        col_base = np.concatenate([[0], np.cumsum(R_t)]).astype(np.int64)
        calls = []
        t = 0
        while t < n_tiles:
            R = R_t[t]
            tb = t
            while tb < n_tiles and R_t[tb] == R:
                tb += 1
            T = max(1, min(TMAX, MF // R))
            while t < tb:
                Tc = min(T, tb - t)
                calls.append((t, Tc, R, int(col_base[t])))
                t += Tc
        sub_meta[s].update(n_tiles=n_tiles, R_t=R_t, col_base=col_base,
                           NR=int(col_base[-1]), calls=calls, L=L)

    tile_base = np.concatenate([[0], np.cumsum([m["n_tiles"] for m in sub_meta])])
    gidx_base = np.concatenate([[0], np.cumsum([m["HCH"] * 128 for m in sub_meta])])
    fidx_base = np.concatenate([[0], np.cumsum([m["NR"] * 128 for m in sub_meta])])
    GT = int(gidx_base[-1])
    FT = int(fidx_base[-1])
    n_tiles_tot = int(tile_base[-1])

    meta = dict(N=N, S=S, NSUB=NSUB, CORE_ROWS=CORE_ROWS, sub_lens=sub_lens,
                sub_meta=sub_meta, tile_base=tile_base, gidx_base=gidx_base,
                fidx_base=fidx_base, GT=GT, FT=FT, n_tiles_tot=n_tiles_tot)

    in_maps = []
    perms = []   # per core: global out row (in renumbered space) per (sub tile rank)
    for c in range(n_cores):
        tab_lo = c * S - MARG
        table = np.zeros((CORE_ROWS, CHW), dtype=BF16)
        glo = max(0, tab_lo)
        ghi = min(N, tab_lo + CORE_ROWS)
        table[glo - tab_lo:ghi - tab_lo, :C] = feats_r[glo:ghi].astype(BF16)

        gidx = np.zeros(GT, np.int64)
        fidx = np.zeros(FT, np.int64)
        core_perm = []
        for s in range(NSUB):
            sm = sub_meta[s]
            HCH = sm["HCH"]
            base_s = s * SUB
            runs = core_sub_runs[c][s]
            # gather idxs + slot map: contribution i of (k) -> slot (col, p)
            g_sub = np.zeros(HCH * 128, np.int64)   # idx j = col*128 + p
            slot_of = {}
            all_d = []
            all_slot = []
            for k in range(KVOL):
                d_loc, src_loc = runs[k]
                Lk = len(d_loc)
                if Lk:
                    j = np.arange(Lk)
                    cols = sm["chunk_start_k"][k] + j // 128
                    ps = j % 128
                    rel = src_loc - base_s
                    assert rel.min() >= 0 and rel.max() < WIN, (c, s, k, rel.min(), rel.max())
                    g_sub[cols * 128 + ps] = rel
                    all_d.append(d_loc)
                    all_slot.append(ps * HCH + cols)   # h row id
            gidx[gidx_base[s]:gidx_base[s + 1]] = g_sub

            # fold: count-sort dests within sub (per core), fill fold idxs
            L = sm["L"]
            counts = np.zeros(L, np.int64)
            ad = np.concatenate(all_d)
            aslot = np.concatenate(all_slot)
            np.add.at(counts, ad, 1)
            order_sub = np.argsort(-counts, kind="stable")
            rank = np.empty(L, np.int64)
            rank[order_sub] = np.arange(L)
            pr = rank[ad]
            o2 = np.argsort(pr, kind="stable")
            sr = pr[o2]
            sh = aslot[o2]
            grp_start = np.searchsorted(sr, np.arange(L))
            r_idx = np.arange(len(sr)) - grp_start[sr]
            t_of = sr // 128
            p_of = sr % 128
            R_arr = np.array(sm["R_t"])
            assert (r_idx < R_arr[t_of]).all()
            col = sm["col_base"][t_of] + r_idx
            f_sub = np.empty(sm["NR"] * 128, np.int64)
            # pads -> zero chunk row of same partition: p*HCH + ZC
            pcol = np.arange(sm["NR"] * 128)
            f_sub[:] = (pcol % 128) * HCH + sm["ZC"]
            f_sub[col * 128 + p_of] = sh
            fidx[fidx_base[s]:fidx_base[s + 1]] = f_sub
            core_perm.append(c * S + base_s + order_sub)

        in_maps.append({
            "table": table,
            "w_sb": w_sb,
            "gidx": pack_idx(gidx),
            "fidx": pack_idx(fidx),
        })
        perms.append(core_perm)

    return in_maps, perms, meta, order


def build_program(n_cores, meta):
    import os
    KSUBS = int(os.environ.get("KSUBS", "0")) or None      # limit #subs
    KNOFOLD = os.environ.get("KNOFOLD", "0") == "1"        # skip fold phase
    KNOGATH = os.environ.get("KNOGATH", "0") == "1"        # skip gather+compute
    NSUB = meta["NSUB"]
    sub_meta = meta["sub_meta"]
    CORE_ROWS = meta["CORE_ROWS"]
    n_tiles_tot = meta["n_tiles_tot"]

    nc = bacc.Bacc("TRN2", target_bir_lowering=False, debug=False,
                   num_devices=n_cores, num_swdge_queues=4)

    table = nc.dram_tensor("table", [CORE_ROWS, CHW], mybir.dt.bfloat16,
                           kind="ExternalInput").ap()
    w_in = nc.dram_tensor("w_sb", [128, KVOL * C], mybir.dt.bfloat16,
                          kind="ExternalInput").ap()
    gidx_d = nc.dram_tensor("gidx", [128, meta["GT"] // 16], mybir.dt.int16,
                            kind="ExternalInput").ap()
    fidx_d = nc.dram_tensor("fidx", [128, meta["FT"] // 16], mybir.dt.int16,
                            kind="ExternalInput").ap()
    out = nc.dram_tensor("out", [128, n_tiles_tot * C], mybir.dt.float32,
                         kind="ExternalOutput").ap()

    with tile.TileContext(nc) as tc, ExitStack() as ctx:
        dram = ctx.enter_context(tc.tile_pool(name="dram", bufs=2, space="DRAM"))

        wpool = ctx.enter_context(tc.tile_pool(name="w", bufs=1))
        w_t = wpool.tile([128, KVOL * C], mybir.dt.bfloat16)
        nc.sync.dma_start(out=w_t[:], in_=w_in[:])
        ident = wpool.tile([128, 128], mybir.dt.bfloat16)
        make_identity(nc, ident[:])

        gip = ctx.enter_context(tc.tile_pool(name="gi", bufs=5))
        fip = ctx.enter_context(tc.tile_pool(name="fi", bufs=5))
        gp = ctx.enter_context(tc.tile_pool(name="G", bufs=6))
        xp = ctx.enter_context(tc.tile_pool(name="X", bufs=8))
        hp = ctx.enter_context(tc.tile_pool(name="H", bufs=4))
        sp = ctx.enter_context(tc.tile_pool(name="slots", bufs=4))
        op = ctx.enter_context(tc.tile_pool(name="outp", bufs=4))
        psx = ctx.enter_context(tc.tile_pool(name="psx", bufs=4, space="PSUM"))
        psh = ctx.enter_context(tc.tile_pool(name="psh", bufs=4, space="PSUM"))

        for s in range(NSUB if KSUBS is None else min(NSUB, KSUBS)):
            sm = sub_meta[s]
            HCH = sm["HCH"]
            chunk_k = sm["chunk_k"]
            HCH_REAL = sm["HCH_REAL"]
            base_s = s * SUB

            h_sub = dram.tile([128 * HCH, CHW], mybir.dt.bfloat16)

            # groups entirely past the zero chunk are never referenced
            g_last = cdiv(HCH_REAL + 1, QG)
            for q0 in ([] if KNOGATH else range(0, g_last * QG, QG)):
                # gather only real chunk columns (zero/pad cols are memset)
                nreal = max(0, min(QG, HCH_REAL - q0))
                ncg = max(nreal, 1)
                gi = gip.tile([128, QG * 8], mybir.dt.int16)
                c0 = (int(meta["gidx_base"][s]) + q0 * 128) // 16
                nc.sync.dma_start(out=gi[:, :ncg * 8],
                                  in_=gidx_d[:, c0:c0 + ncg * 8])
                gbuf = gp.tile([128, QG * CHW], mybir.dt.bfloat16)
                nc.gpsimd.dma_gather(
                    out_ap=gbuf[:, :ncg * CHW].rearrange(
                        "p (m c) -> p m c", c=CHW),
                    in_ap=table[base_s:base_s + WIN, :],
                    idxs_ap=gi[:, :ncg * 8],
                    num_idxs=ncg * 128,
                    num_idxs_reg=ncg * 128,
                    elem_size=CHW,
                    single_packet=False,
                )
                h_t = hp.tile([128, QG * CHW], mybir.dt.bfloat16)
                if q0 + QG > HCH_REAL:
                    # group contains pad/zero chunks: zero the whole tile so
                    # fold pads (and H-write reads) see defined zeros
                    nc.vector.memset(h_t[:], 0.0)
                for q in range(QG):
                    colc = q0 + q
                    if colc >= HCH_REAL:
                        continue
                    k = chunk_k[colc]
                    x_ps = psx.tile([64, 128], mybir.dt.bfloat16)
                    nc.tensor.transpose(
                        out=x_ps[:], in_=gbuf[:, q * CHW:q * CHW + C],
                        identity=ident[:])
                    x_t = xp.tile([64, 128], mybir.dt.bfloat16)
                    nc.vector.tensor_copy(out=x_t[:], in_=x_ps[:])
                    h_ps = psh.tile([128, C], mybir.dt.float32)
                    nc.tensor.matmul(
                        out=h_ps[:],
                        lhsT=x_t[:],
                        rhs=w_t[0:64, k * C:(k + 1) * C],
                        start=True, stop=True,
                    )
                    nc.scalar.activation(
                        h_t[:, q * CHW:q * CHW + C], h_ps[:],
                        mybir.ActivationFunctionType.Copy,
                    )
                # H rows: slot (col, p) -> row p*HCH + col; this group: cols [q0, q0+QG)
                nc.sync.dma_start(
                    out=h_sub[:].rearrange(
                        "(p q) c -> p q c", q=HCH)[:, q0:q0 + QG, :],
                    in_=h_t[:].rearrange("p (q c) -> p q c", c=CHW),
                )

            for (t0, T, R, col0) in ([] if KNOFOLD else sm["calls"]):
                ncols = T * R
                fi = fip.tile([128, MF * 8], mybir.dt.int16)
                c0 = (int(meta["fidx_base"][s]) + col0 * 128) // 16
                nc.sync.dma_start(out=fi[:, :ncols * 8],
                                  in_=fidx_d[:, c0:c0 + ncols * 8])
                slots = sp.tile([128, MF * CHW], mybir.dt.bfloat16)
                nc.gpsimd.dma_gather(
                    out_ap=slots[:, :ncols * CHW].rearrange(
                        "p (m c) -> p m c", c=CHW),
                    in_ap=h_sub[:, :],
                    idxs_ap=fi[:, :ncols * 8],
                    num_idxs=ncols * 128,
                    num_idxs_reg=ncols * 128,
                    elem_size=CHW,
                    single_packet=False,
                )
                out_t = op.tile([128, TMAX * C], mybir.dt.float32)
                sl4 = slots[:, :ncols * CHW].rearrange(
                    "p (t r c) -> p t c r", r=R, c=CHW)
                nc.vector.tensor_reduce(
                    out=out_t[:, :T * C].rearrange("p (t c) -> p t c", c=C),
                    in_=sl4[:, :, 0:C, :],
                    axis=mybir.AxisListType.X,
                    op=mybir.AluOpType.add,
                )
                tb = int(meta["tile_base"][s])
                nc.sync.dma_start(
                    out=out[:, (tb + t0) * C:(tb + t0 + T) * C],
                    in_=out_t[:, :T * C])

    nc.compile()
    # spread gathers across the 4 SWDGE queues (Q7 core pairs) consistently
    # with the DMASW sem lane Tile assigned: queue = lane % 4 keeps each sem
    # locked to a single queue while 4 Q7 pairs generate descriptors in parallel
    import re
    nq = 0
    for bb in nc.main_func.blocks:
        for inst in bb.instructions:
            if isinstance(inst, mybir.InstDMAGatherAnt):
                m = re.match(r"DMASW(\d+)_", inst.sync_info.on_update[0].ant_name)
                if m:
                    inst.queue_num = int(m.group(1)) % 4
                    nq += 1
    return nc


def assemble_output(results, perms, meta, order, n_cores):
    S = meta["S"]
    N = meta["N"]
    sub_meta = meta["sub_meta"]
    out = np.empty((N, C), np.float32)
    for c in range(n_cores):
        rows = results[c]["out"]  # [128, n_tiles_tot*C]
        for s in range(meta["NSUB"]):
            sm = sub_meta[s]
            tb = int(meta["tile_base"][s])
            arr = rows[:, tb * C:(tb + sm["n_tiles"]) * C]
            arr = arr.reshape(128, sm["n_tiles"], C).transpose(1, 0, 2).reshape(-1, C)
            dest_rows = perms[c][s]          # renumbered-space row ids
            out[order[dest_rows]] = arr[:len(dest_rows)]
    return out


N_CORES = 8
LAST_EXEC_TIME_NS = None

_CACHE = {}


def kernel(feats, weight, kernel_map):
    """Full-input entry point: shard, run on 8 NeuronCores, unshard."""
    global LAST_EXEC_TIME_NS
    import os
    from concourse import bass_utils

    feats = np.asarray(feats)
    weight = np.asarray(weight)
    kernel_map = np.asarray(kernel_map)

    in_maps, perms, meta, order = host_prep(feats, weight, kernel_map, N_CORES)
    key = (meta["GT"], meta["FT"], meta["n_tiles_tot"],
           tuple(m["HCH"] for m in meta["sub_meta"]),
           tuple(tuple(m["R_t"]) for m in meta["sub_meta"]))
    if key in _CACHE:
        nc = _CACHE[key]
    else:
        nc = build_program(N_CORES, meta)
        _CACHE[key] = nc

    trace = os.environ.get("BASS_KERNEL_TRACE", "0") == "1"
    res = bass_utils.run_bass_kernel_spmd(
        nc, in_maps, core_ids=list(range(N_CORES)), trace=trace)
    LAST_EXEC_TIME_NS = res.exec_time_ns
    return assemble_output(res.results, perms, meta, order, N_CORES)
